# revision 40
# baseline (speedup 1.0000x reference)
"""Multi-head causal self-attention (q=k=v bug faithful) on 8 trn2 cores.

Sharding: 24 (batch, head) jobs -> 3 heads per core. Core c: batch c//4,
heads (c%4)*3 .. +3. Each core computes its heads' attention outputs and a
partial output-projection Z^T = sum_h O_h @ Wout_slice_h  (shape [768, 4096],
bias folded in via a ones row on one core per batch). Host: sum the 4 bf16
partials per batch in f32, transpose to [4096, 768].

Device algorithm per core (bf16 matmuls, fp32 PSUM accumulation):
  1. Q^T[h] = (sqrt(s)*Wq_h) @ X^T  via 6 K-chunks of 128  (s = 1/sqrt(768));
     heads 0,1 packed into one [128,IG] psum (M=64+64), head 2 separate.
     Q natural (qn) via PE-mode transpose of [64,128] blocks.
  2. flash-style, i-windows of 1024, j-blocks of 128 (causal-skipped):
       S^T[jb, i] = Q^T[:,jb-block].T @ Q^T[:,i-span]   (PSUM, one [128,1024]
                    f32 tile per jb; diagonal blocks trimmed to the causal
                    span; two 512-wide moving halves per jb)
       P^T = exp(S^T)  (ScalarE, one FD<=1024 activation per jb,
                        PSUM->SBUF bf16; no max-subtraction -- scores are
                        bounded ~+-3.5 for randn inputs)
       diag band masked by upper-tri 0/1 mask multiply (DVE, bf16)
       [O | denom]^T += [Q[jb]|1].T @ P^T     (single-buffered [65,1024]
                        PSUM accumulator, drained to SBUF in one copy)
     normalize: O^T *= 1/denom (DVE reciprocal_approx_fast + gpsimd
     partition_broadcast + DVE multiply)
  3. Z^T[oc, i] = [Wout_slice | bias].T @ [O_cat | 1]^T  (6 out-chunks of
     128; ones row of ot2 supplies the bias term), stored bf16.

Hard-won constraints on this part (each cost a failed experiment):
  * POWER is the binding limit, with hysteresis: schedules that run the PE
    array denser (row-packed K=64 pairs via tile_position, xbar-DMA
    transpose bursts, tighter phase overlap) trip a power/thermal throttle
    that drops EVERY engine clock 1.2-2x, sometimes stickily for the rest
    of the kernel (measured 400-670us vs 371us for the same math).
  * Every matmul pays its own ~105-120ns weight-swap bubble: walrus emits
    LDWEIGHTS per matmul even for identical consecutive lhsT, and a
    weight swap cannot overlap the previous matmul's drain.
  * Custom-DVE ops (reciprocal_approx_*) silently misread operands that
    do not start at partition 0 (and PSUM operands) -- stage to a
    partition-0 SBUF tile first.
  * Concurrent row-group-split accumulation chains corrupt PSUM; xbar
    transposes need fully contiguous dests; nc.sync DMA issues queue
    FIFO behind in-flight xbar transposes (~1.2us each).
"""

import os

import numpy as np

B, L, D, H, HS = 2, 4096, 768, 12, 64
NCORES = 8
HPC = 3  # heads per core
IG = 512  # i-group width
NIG = L // IG
SCALE = 1.0 / np.sqrt(np.float32(D))
SQS = np.sqrt(SCALE).astype(np.float32)  # folded into Wq (and undone in Wout)

_cached = {}


def _build_program():
    import concourse.bass as bass
    import concourse.mybir as mybir
    import concourse.tile as tile
    from concourse import bacc
    from concourse.masks import make_identity, make_upper_triangular

    f32 = mybir.dt.float32
    bf16 = mybir.dt.bfloat16
    Exp = mybir.ActivationFunctionType.Exp
    Copy = mybir.ActivationFunctionType.Copy

    nc = bacc.Bacc(
        "TRN2",
        target_bir_lowering=False,
        debug=False,
        enable_asserts=False,
        num_devices=NCORES,
    )

    xT = nc.dram_tensor("xT", [D, L], bf16, kind="ExternalInput").ap()
    wqT = nc.dram_tensor("wqT", [D, HPC * HS], bf16, kind="ExternalInput").ap()
    # rows 0:192 = Wout slice (transposed, /SQS), row 192 = bias (one core/batch)
    wout = nc.dram_tensor("wout", [HPC * HS + 1, D], bf16, kind="ExternalInput").ap()
    zT = nc.dram_tensor("zT", [D, L], bf16, kind="ExternalOutput").ap()

    xT_r = xT.rearrange("(c p) i -> p c i", p=128)  # [128, 6, L]
    zT_r = zT.rearrange("(c p) i -> c p i", p=128)  # [6, 128, L]

    with tile.TileContext(nc) as tc:
        with (
            tc.tile_pool(name="consts", bufs=1) as consts,
            tc.tile_pool(name="persist", bufs=1) as persist,
        ):
            # ---- constants ----
            wq_sb = consts.tile([128, 6, HPC * HS], bf16)
            nc.sync.dma_start(out=wq_sb, in_=wqT.rearrange("(c p) m -> p c m", p=128))
            wout0_sb = consts.tile([128, D], bf16)
            nc.sync.dma_start(out=wout0_sb, in_=wout[0:128, :])
            wout1_sb = consts.tile([65, D], bf16)  # rows 0:64 wout, row 64 bias
            nc.sync.dma_start(out=wout1_sb, in_=wout[128:193, :])
            # keep mask[p, t] = 1.0 where t >= p else 0.0
            trimask = consts.tile([128, 128], bf16)
            make_upper_triangular(nc, trimask, val=1.0, diag=True)
            ident64 = consts.tile([64, 64], bf16)
            make_identity(nc, ident64)

            # ---- persistent per-head state ----
            # Q^T per head [64, L]; Q natural in qn (col 64 = ones for the
            # softmax-denominator row of the av matmul).  NOTE: row-packing
            # two K=64 score matmuls into both PE row-groups was tried and
            # REVERTED: the doubled array activity trips a power-state
            # downclock (-17% on every engine) that costs more than it saves.
            qts = [persist.tile([64, L], bf16, name=f"qt{h}") for h in range(HPC)]
            qn = persist.tile([128, 32, HPC, 65], bf16)
            ot01 = persist.tile([128, L], bf16)  # O^T heads 0,1
            ot2 = persist.tile([65, L], bf16)  # O^T head 2 + ones row (bias)
            nc.vector.memset(qn[:, :, :, 64:65], 1.0)
            nc.vector.memset(ot2[64:65, :], 1.0)

            def ot_h(h, js, je):
                if h < 2:
                    return ot01[h * 64 : (h + 1) * 64, js:je]
                return ot2[0:64, js:je]

            # ---- phase 1: Q^T projection + Q natural via PE transpose.
            # (xbar transposes + denser overlap were tried and REVERTED:
            # the extra concurrent power trips a STICKY half-clock thermal
            # throttle that costs far more than the ~20us they save) ----
            with (
                tc.tile_pool(name="xin", bufs=2) as xin,
                tc.tile_pool(name="qps", bufs=1, space="PSUM") as qps,
                tc.tile_pool(name="tps", bufs=3, space="PSUM") as tps,
            ):
                xts = {}

                def load_xt(ig):
                    i0 = ig * IG
                    xt = xin.tile([128, 6, IG], bf16, tag="xt", name="xt")
                    xts[ig] = xt
                    nc.sync.dma_start(out=xt, in_=xT_r[:, :, i0 : i0 + IG])

                load_xt(0)
                load_xt(1)
                for igp1 in range(NIG // 2):
                    iga, igb = 2 * igp1, 2 * igp1 + 1
                    xta, xtb = xts.pop(iga), xts.pop(igb)
                    # two i-groups per pass: each wq chunk's weights serve
                    # two consecutive matmuls (second one skips the
                    # weight-swap bubble); heads 0,1 packed (M=128),
                    # head 2 alone (M=64)
                    qpa = qps.tile([128, IG], f32, tag="qp01", name="qpa")
                    qpb = qps.tile([128, IG], f32, tag="qp01", name="qpb")
                    for c in range(6):
                        for qp, xt in ((qpa, xta), (qpb, xtb)):
                            nc.tensor.matmul(
                                qp,
                                lhsT=wq_sb[:, c, 0:128],
                                rhs=xt[:, c, :],
                                start=(c == 0),
                                stop=(c == 5),
                            )
                    qp2a = qps.tile([64, IG], f32, tag="qp2", name="qp2a")
                    qp2b = qps.tile([64, IG], f32, tag="qp2", name="qp2b")
                    for c in range(6):
                        for qp, xt in ((qp2a, xta), (qp2b, xtb)):
                            nc.tensor.matmul(
                                qp,
                                lhsT=wq_sb[:, c, 128:192],
                                rhs=xt[:, c, :],
                                start=(c == 0),
                                stop=(c == 5),
                            )
                    if igb + 1 < NIG:
                        load_xt(igb + 1)
                    if igb + 2 < NIG:
                        load_xt(igb + 2)
                    for ig, qp01, qp2 in ((iga, qpa, qp2a), (igb, qpb, qp2b)):
                        i0 = ig * IG
                        nc.vector.tensor_copy(
                            out=qts[0][:, i0 : i0 + IG], in_=qp01[0:64, :]
                        )
                        nc.vector.tensor_copy(
                            out=qts[1][:, i0 : i0 + IG], in_=qp01[64:128, :]
                        )
                        nc.vector.tensor_copy(
                            out=qts[2][:, i0 : i0 + IG], in_=qp2
                        )
                        # Q natural: PE-transpose each [64,128] block
                        for r in range(4):
                            jb = 4 * ig + r
                            for h in range(HPC):
                                trp = tps.tile([128, 64], bf16, tag="trp")
                                nc.tensor.transpose(
                                    trp,
                                    qts[h][:, jb * 128 : (jb + 1) * 128],
                                    ident64,
                                )
                                nc.vector.tensor_copy(
                                    out=qn[:, jb, h, 0:64], in_=trp
                                )

            # ---- phase 2: attention + output projection ----
            # i-groups of 1024: each j-block's stationary operand (scores:
            # Q^T block, av: Q natural) is loaded once and streamed over two
            # 512-wide moving halves -- consecutive same-weight matmuls skip
            # the weight-swap drain bubble (~105-120ns each).  One FD-1024
            # exp per j-block keeps ScalarE instruction overhead low.
            # PSUM: sc 2x2 banks + av 1x2 + zt 2x1 = 8 exactly (av is
            # single-buffered: a full copy to SBUF frees it for the next
            # head and the normalize runs from SBUF).
            IGP = 2 * IG
            with (
                tc.tile_pool(name="scps", bufs=2, space="PSUM") as scps,
                tc.tile_pool(name="avps", bufs=1, space="PSUM") as avps,
                tc.tile_pool(name="ztps", bufs=2, space="PSUM") as ztps,
                tc.tile_pool(name="ptp", bufs=4) as ptp,
                tc.tile_pool(name="ztb", bufs=4) as ztb,
                tc.tile_pool(name="nrm", bufs=2) as nrm,
            ):
                for igp in range(L // IGP):
                    i0 = igp * IGP
                    jb_max = 8 * (igp + 1)
                    last_a = 8 * igp + 3  # last j-block touching cols [0:512)

                    def spans(sr):
                        out = []
                        for lo in (0, IG):
                            s0 = max(sr, lo)
                            if s0 < lo + IG:
                                out.append((s0, lo + IG))
                        return out

                    for h in range(HPC):
                        av = avps.tile([65, IGP], f32, tag="av")
                        scs = {}

                        def emit_scores(jb, h=h, igp=igp, i0=i0):
                            r = jb - 8 * igp
                            sr = 128 * r if r > 0 else 0  # causal trim
                            sc = scps.tile([128, IGP], f32, tag="sc", name="sc")
                            scs[jb] = sc
                            for s0, s1 in spans(sr):
                                nc.tensor.matmul(
                                    sc[:, s0:s1],
                                    lhsT=qts[h][:, jb * 128 : (jb + 1) * 128],
                                    rhs=qts[h][:, i0 + s0 : i0 + s1],
                                    start=True,
                                    stop=True,
                                )

                        # software pipeline: keep TWO j-blocks of scores in
                        # flight ahead of the exp/mask/av consumers so the
                        # in-order PE queue never stalls on the exp semaphore
                        emit_scores(0)
                        emit_scores(1)
                        for jb in range(jb_max):
                            r = jb - 8 * igp
                            sr = 128 * r if r > 0 else 0
                            sc = scs.pop(jb)
                            pt = ptp.tile([128, IGP], bf16, tag="pt", name="pt")
                            nc.scalar.activation(
                                out=pt[:, sr:IGP], in_=sc[:, sr:IGP], func=Exp
                            )
                            if jb + 2 < jb_max:
                                emit_scores(jb + 2)
                            if r >= 0:  # diagonal band: zero out j > i
                                band = slice(sr, sr + 128)
                                nc.vector.tensor_mul(
                                    pt[:, band], pt[:, band], trimask
                                )
                            for s0, s1 in spans(sr):
                                stop = (
                                    jb == (last_a if s1 == IG else jb_max - 1)
                                )
                                nc.tensor.matmul(
                                    av[:, s0:s1],
                                    lhsT=qn[:, jb, h, 0:65],
                                    rhs=pt[:, s0:s1],
                                    start=(jb == 0),
                                    stop=stop,
                                    skip_group_check=True,
                                )
                        # drain av to SBUF in one copy (frees the single
                        # av buffer for the next head), normalize from SBUF
                        asum = nrm.tile([65, IGP], f32, tag="asum")
                        nc.vector.tensor_copy(out=asum, in_=av)
                        # custom-DVE ops misread partition-offset operands:
                        # stage the denom row to a partition-0 tile first
                        dsb = nrm.tile([1, IGP], f32, tag="dsb")
                        nc.vector.tensor_copy(out=dsb, in_=asum[64:65, :])
                        recip = nrm.tile([1, IGP], f32, tag="recip")
                        nc.vector.reciprocal_approx_fast(recip, dsb)
                        rb = nrm.tile([64, IGP], f32, tag="rb")
                        nc.gpsimd.partition_broadcast(rb, recip, channels=64)
                        nc.vector.tensor_mul(
                            ot_h(h, i0, i0 + IGP), asum[0:64, :], rb
                        )
                    for oc in range(6):
                        # both 512-halves of each weight chunk back-to-back
                        # so the second matmul reuses the loaded weights
                        zts = [
                            ztps.tile([128, IG], f32, tag="zt", name="zt")
                            for _ in range(2)
                        ]
                        for half in range(2):
                            nc.tensor.matmul(
                                zts[half],
                                lhsT=wout0_sb[:, oc * 128 : (oc + 1) * 128],
                                rhs=ot01[:, i0 + half * IG : i0 + (half + 1) * IG],
                                start=True,
                                stop=False,
                            )
                        for half in range(2):
                            nc.tensor.matmul(
                                zts[half],
                                lhsT=wout1_sb[:, oc * 128 : (oc + 1) * 128],
                                rhs=ot2[:, i0 + half * IG : i0 + (half + 1) * IG],
                                start=False,
                                stop=True,
                            )
                        zb = ztb.tile([128, IGP], bf16, tag="zb")
                        for half in range(2):
                            nc.vector.tensor_copy(
                                out=zb[:, half * IG : (half + 1) * IG],
                                in_=zts[half],
                            )
                        nc.sync.dma_start(out=zT_r[oc, :, i0 : i0 + IGP], in_=zb)

    nc.compile()
    return nc


def _get_program():
    if "nc" not in _cached:
        _cached["nc"] = _build_program()
    return _cached["nc"]


def _make_in_maps(x, Wq, W_out, b_out):
    import ml_dtypes

    bf16 = ml_dtypes.bfloat16
    x = np.asarray(x, dtype=np.float32)
    Wq = np.asarray(Wq, dtype=np.float32)
    W_out = np.asarray(W_out, dtype=np.float32)
    b_out = np.asarray(b_out, dtype=np.float32)
    in_maps = []
    for c in range(NCORES):
        b = c // (NCORES // B)
        hg = c % (NCORES // B)
        h0 = hg * HPC
        xT = np.ascontiguousarray(x[b].T).astype(bf16)  # [D, L]
        wq = Wq[h0 : h0 + HPC]  # [3, 64, D]
        wqT = (
            np.ascontiguousarray(wq.transpose(2, 0, 1).reshape(D, HPC * HS)) * SQS
        ).astype(bf16)
        wout = np.empty((HPC * HS + 1, D), dtype=np.float32)
        wout[0 : HPC * HS] = W_out[:, h0 * HS : (h0 + HPC) * HS].T / SQS
        wout[HPC * HS] = b_out if hg == 0 else 0.0
        in_maps.append(
            {"xT": xT, "wqT": wqT, "wout": np.ascontiguousarray(wout).astype(bf16)}
        )
    return in_maps


def run(x, Wq, W_out, b_out, trace=False):
    from concourse.bass_utils import run_bass_kernel_spmd

    nc = _get_program()
    in_maps = _make_in_maps(x, Wq, W_out, b_out)
    res = run_bass_kernel_spmd(
        nc, in_maps, core_ids=list(range(NCORES)), trace=trace
    )
    partials = [r["zT"] for r in res.results]  # each [D, L] bf16
    out = np.empty((B, L, D), dtype=np.float32)
    for b in range(B):
        g = NCORES // B
        acc = partials[b * g].astype(np.float32)
        for c in range(b * g + 1, (b + 1) * g):
            acc += partials[c].astype(np.float32)
        out[b] = acc.T
    return out, res


def kernel(x, Wq, W_out, b_out):
    out, _ = run(
        x, Wq, W_out, b_out, trace=bool(int(os.environ.get("KERNEL_TRACE", "0")))
    )
    return out
